# revision 1
# baseline (speedup 1.0000x reference)
# Trainium2 Bass kernel for nn_CrossFrequencyInteraction.
#
# Reference computation (per batch item, two symmetric branches):
#   q = Wq @ x_q;  k = Wk @ x_kv;  v = Wv @ x_kv          (1x1 convs, C=256)
#   out = softmax_n(q) used against ctx = softmax_n(k) @ v^T   (linear attention)
#   inter = Wp @ out;  x_q += inter
#   then training-mode BatchNorm over (B,H,W) on both updated rgb tensors.
#
# Sharding: data-parallel over batch (B=8 -> 1 item per core, 8 cores).
# BN statistics (per-channel sum/sumsq) are AllReduced across cores (2KB).
#
# Key algebraic restructurings (all exact):
#   - b_q, b_k shift softmax inputs by a per-row constant along the softmax
#     axis -> they cancel exactly; skipped.
#   - b_proj is a per-channel constant shift -> absorbed exactly by BN; skipped.
#   - b_v adds b_v[e] to ctx[d,e] (softmax_k sums to 1) -> folded into ctx.
#   - softmax normalizers (1/sum exp) for q and k are per-channel scales that
#     commute through the attention contraction -> folded into the tiny
#     M = Wp . blockdiag(ctx^T) matrix, so attention-out + proj become a
#     single [256,256] @ [256,4096] matmul per branch.
#   - kT/vT are produced directly in transposed layout by using x as the
#     stationary matmul operand (no explicit transposes anywhere); the
#     softmax-k denominators come for free from a ones-column in vT.

import os
import numpy as np

C = 256
N = 4096
NBLK = 2          # channel blocks of 128
NT = 32           # n-tiles of 128 (for transposed convs)
NCH = 8           # n-chunks of 512 (for natural convs)
NH = 4            # heads
HD = 64           # head dim
NCORES = 8
BHW = 8 * 64 * 64  # BN reduction count
EPS = 1e-5

_CACHE = {}


def _build():
    import concourse.bass as bass
    import concourse.bacc as bacc
    import concourse.tile as tile
    from concourse import mybir
    from contextlib import ExitStack

    F32 = mybir.dt.float32
    F32R = mybir.dt.float32r
    BF16 = mybir.dt.bfloat16
    OP = mybir.AluOpType
    AF = mybir.ActivationFunctionType
    AX = mybir.AxisListType

    nc = bacc.Bacc("TRN2", num_devices=NCORES)

    xq_d = [nc.dram_tensor(n_, [C, N], F32, kind="ExternalInput")
            for n_ in ("xq1", "xq2")]
    xkv_d = [nc.dram_tensor(n_, [C, N], F32, kind="ExternalInput")
             for n_ in ("xkv1", "xkv2")]
    # wt: [256, 8*256] = [Wq1^T|Wk1^T|Wv1^T|Wp1^T|Wq2^T|Wk2^T|Wv2^T|Wp2^T]
    wt_d = nc.dram_tensor("wt", [C, 8 * 256], BF16, kind="ExternalInput")
    # wp: [64, 8*256]; block (b*4+h) = Wp_b^T[h*64:(h+1)*64, :]
    wp_d = nc.dram_tensor("wp", [64, 8 * 256], BF16, kind="ExternalInput")
    # bp: [128, 8] cols = (bv1_b0, bv1_b1, bv2_b0, bv2_b1, g_b0, g_b1, be_b0, be_b1)
    bp_d = nc.dram_tensor("bp", [128, 8], F32, kind="ExternalInput")
    # bh: [64, 8]; col (b*4+h) = b_v[h*64:(h+1)*64] for branch b
    bh_d = nc.dram_tensor("bh", [64, 8], F32, kind="ExternalInput")
    out_d = [nc.dram_tensor(n_, [C, N], F32, kind="ExternalOutput")
             for n_ in ("out1", "out2")]

    with ExitStack() as ctx:
        tc = ctx.enter_context(tile.TileContext(nc))
        const = ctx.enter_context(tc.tile_pool(name="const", bufs=1))
        xqp = ctx.enter_context(tc.tile_pool(name="xqp", bufs=1))
        xkvp = ctx.enter_context(tc.tile_pool(name="xkvp", bufs=1))
        eqp = ctx.enter_context(tc.tile_pool(name="eqp", bufs=1))
        ekp = ctx.enter_context(tc.tile_pool(name="ekp", bufs=6))
        vtp = ctx.enter_context(tc.tile_pool(name="vtp", bufs=6))
        misc = ctx.enter_context(tc.tile_pool(name="misc", bufs=1))
        scr = ctx.enter_context(tc.tile_pool(name="scr", bufs=2))
        bigp = ctx.enter_context(tc.tile_pool(name="bigp", bufs=4, space="PSUM"))
        smallp = ctx.enter_context(tc.tile_pool(name="smallp", bufs=4, space="PSUM"))
        dramp = ctx.enter_context(tc.tile_pool(name="dramp", bufs=1, space="DRAM"))

        # ---- constants ----
        wt_sb = []
        for k in range(NBLK):
            w = const.tile([128, 8 * 256], BF16, name=f"wt{k}", tag=f"wt{k}")
            nc.sync.dma_start(out=w, in_=wt_d[k * 128:(k + 1) * 128, :])
            wt_sb.append(w)
        wp_sb = const.tile([64, 8 * 256], BF16, name="wp", tag="wp")
        nc.sync.dma_start(out=wp_sb, in_=wp_d[:, :])
        bp_sb = const.tile([128, 8], F32, name="bp", tag="bp")
        nc.sync.dma_start(out=bp_sb, in_=bp_d[:, :])
        bh_sb = const.tile([64, 8], F32, name="bh", tag="bh")
        nc.sync.dma_start(out=bh_sb, in_=bh_d[:, :])
        ones_col = const.tile([128, 1], BF16, name="ones_col", tag="ones_col")
        nc.vector.memset(ones_col, 1.0)

        expq = [eqp.tile([128, N], BF16, name=f"expq{k}", tag=f"expq{k}")
                for k in range(NBLK)]

        rg = [list(range(NCORES))]

        ablate = os.environ.get("KERNEL_ABLATE", "")

        def branch(b):
            wofs = 4 * b
            # ---- load inputs ----
            xkv = []
            for k in range(NBLK):
                t = xkvp.tile([128, N], BF16, name=f"xkv{k}_b{b}", tag=f"xkv{k}")
                nc.gpsimd.dma_start(out=t, in_=xkv_d[b][k * 128:(k + 1) * 128, :])
                xkv.append(t)
            xq = []
            xqb = []
            for k in range(NBLK):
                t = xqp.tile([128, N], F32, name=f"xq{k}_b{b}", tag=f"xq{k}_b{b}")
                nc.sync.dma_start(out=t, in_=xq_d[b][k * 128:(k + 1) * 128, :])
                xq.append(t)
                tb = xqp.tile([128, N], BF16, name=f"xqb{k}_b{b}", tag=f"xqb{k}")
                nc.vector.tensor_copy(tb, t)
                xqb.append(tb)

            # ---- KV phase: kT|vT transposed convs + exp(k) + ctx accumulation
            pctx = smallp.tile([128, 256], F32, name=f"pctx_b{b}", tag="small")
            pden = smallp.tile([1, 256], F32, name=f"pden_b{b}", tag="small")
            wkv0 = (wofs + 1) * 256
            for t in range(NT):
                pkv = bigp.tile([128, 512], F32, name=f"pkv_b{b}_{t}", tag="big")
                for k in range(NBLK):
                    nc.tensor.matmul(
                        pkv,
                        lhsT=xkv[k][:, t * 128:(t + 1) * 128],
                        rhs=wt_sb[k][:, wkv0:wkv0 + 512],
                        start=(k == 0), stop=(k == NBLK - 1),
                    )
                ek = ekp.tile([128, 256], BF16, name=f"ek_b{b}_{t}", tag="ek")
                nc.scalar.activation(ek, pkv[:, 0:256], AF.Exp)
                vt = vtp.tile([128, 256], BF16, name=f"vt_b{b}_{t}", tag="vt")
                nc.vector.tensor_copy(vt, pkv[:, 256:512])
                for h in range(NH):
                    nc.tensor.matmul(
                        pctx[0:HD, h * HD:(h + 1) * HD],
                        lhsT=vt[:, h * HD:(h + 1) * HD],
                        rhs=ek[:, h * HD:(h + 1) * HD],
                        start=(t == 0), stop=(t == NT - 1),
                        skip_group_check=True,
                    )
                nc.tensor.matmul(
                    pden, lhsT=ones_col, rhs=ek,
                    start=(t == 0), stop=(t == NT - 1),
                    skip_group_check=True,
                )

            # ---- ctx eviction (+ b_v fold) and softmax-k denominators ----
            ctxT = misc.tile([64, 256], BF16, name=f"ctxT_b{b}", tag="ctxT")
            for h in range(NH):
                nc.vector.tensor_scalar(
                    ctxT[:, h * HD:(h + 1) * HD],
                    pctx[0:HD, h * HD:(h + 1) * HD],
                    bh_sb[:, b * 4 + h:b * 4 + h + 1],
                    None, OP.add)
            drow = misc.tile([1, 256], F32, name=f"drow_b{b}", tag="drow")
            nc.vector.tensor_copy(drow, pden)
            denT = misc.tile([128, 2], F32, name=f"denT_b{b}", tag="denT")
            for k in range(NBLK):
                nc.sync.dma_start(out=denT[:, k:k + 1],
                                  in_=drow[0:1, k * 128:(k + 1) * 128])
            if ablate.endswith("kv"):
                ctf = misc.tile([64, 256], F32, name=f"ctf_b{b}", tag="ctf")
                nc.vector.tensor_copy(ctf, ctxT)
                nc.sync.dma_start(out=out_d[b][0:64, 0:256], in_=ctf)
                nc.sync.dma_start(out=out_d[b][64:128, 0:2], in_=denT[0:64, :])
                return

            # ---- Q phase: natural conv + exp with accumulated row sums ----
            sqp = misc.tile([128, NBLK, NCH], F32, name=f"sqp_b{b}", tag="sqp")
            wq0 = wofs * 256
            for k in range(NBLK):
                for j in range(NCH):
                    pq = bigp.tile([128, 512], F32, name=f"pq_b{b}_{k}_{j}", tag="big")
                    for kk in range(NBLK):
                        nc.tensor.matmul(
                            pq,
                            lhsT=wt_sb[kk][:, wq0 + k * 128:wq0 + k * 128 + 128],
                            rhs=xqb[kk][:, j * 512:(j + 1) * 512],
                            start=(kk == 0), stop=(kk == NBLK - 1),
                        )
                    nc.scalar.activation(
                        expq[k][:, j * 512:(j + 1) * 512], pq, AF.Exp,
                        accum_out=sqp[:, k, j:j + 1])

            # ---- normalization factor: 1 / (denom_k * sum_q) ----
            sq2 = misc.tile([128, 2], F32, name=f"sq2_b{b}", tag="sq2")
            for k in range(NBLK):
                nc.vector.reduce_sum(sq2[:, k:k + 1], sqp[:, k, :], axis=AX.X)
            fde = misc.tile([128, 2], F32, name=f"fde_b{b}", tag="fde")
            nc.vector.tensor_mul(fde, denT, sq2)
            fac = misc.tile([128, 2], F32, name=f"fac_b{b}", tag="fac")
            nc.vector.reciprocal(fac, fde)

            # ---- M^T: per-head Wp . ctx^T with folded normalization ----
            mt_sb = []
            for p in range(2):
                pmt = smallp.tile([128, 256], F32, name=f"pmt_b{b}_{p}", tag="small")
                for hh in range(2):
                    h = p * 2 + hh
                    nc.tensor.matmul(
                        pmt[hh * 64:(hh + 1) * 64, :],
                        lhsT=ctxT[:, h * HD:(h + 1) * HD],
                        rhs=wp_sb[:, (b * 4 + h) * 256:(b * 4 + h + 1) * 256],
                        start=True, stop=True,
                        tile_position=(0, hh * 64),
                    )
                mt = misc.tile([128, 256], BF16, name=f"mt_b{b}_{p}", tag=f"mt{p}")
                nc.vector.tensor_scalar(mt, pmt, fac[:, p:p + 1], None, OP.mult)
                mt_sb.append(mt)
            if ablate.endswith("q"):
                for p in range(2):
                    mtf = misc.tile([128, 256], F32, name=f"mtf_b{b}_{p}", tag="mtf")
                    nc.vector.tensor_copy(mtf, mt_sb[p])
                    nc.sync.dma_start(out=out_d[b][p * 128:(p + 1) * 128, 0:256], in_=mtf)
                return

            # ---- inter = M^T.T @ expq, fused residual + BN stats ----
            bst = misc.tile([128, NBLK, NCH, 6], F32, name=f"bst_b{b}", tag="bst")
            for k in range(NBLK):
                for j in range(NCH):
                    pi = bigp.tile([128, 512], F32, name=f"pi_b{b}_{k}_{j}", tag="big")
                    for kk in range(NBLK):
                        nc.tensor.matmul(
                            pi,
                            lhsT=mt_sb[kk][:, k * 128:(k + 1) * 128],
                            rhs=expq[kk][:, j * 512:(j + 1) * 512],
                            start=(kk == 0), stop=(kk == NBLK - 1),
                        )
                    xs = xq[k][:, j * 512:(j + 1) * 512]
                    nc.vector.scalar_tensor_tensor(
                        xs, pi, 1.0, xs, OP.mult, OP.add)
                    nc.vector.bn_stats(bst[:, k, j, :], xs)

            if ablate.endswith("i"):
                for k in range(NBLK):
                    nc.sync.dma_start(out=out_d[b][k * 128:(k + 1) * 128, :],
                                      in_=xq[k])
                return

            # ---- BN stats allreduce ----
            cc_sb = misc.tile([128, 4], F32, name=f"cc_b{b}", tag=f"cc{b}")
            mv = misc.tile([128, NBLK, 2], F32, name=f"mv_b{b}", tag="mv")
            for k in range(NBLK):
                nc.vector.bn_aggr(mv[:, k, :], bst[:, k, :, :])
                # sum = mean * N ; sumsq = (var + mean^2) * N   (per-core N)
                nc.vector.tensor_scalar(
                    cc_sb[:, 2 * k:2 * k + 1], mv[:, k, 0:1], float(N), None, OP.mult)
                nc.vector.scalar_tensor_tensor(
                    cc_sb[:, 2 * k + 1:2 * k + 2], mv[:, k, 0:1],
                    mv[:, k, 0:1], mv[:, k, 1:2], OP.mult, OP.add)
                nc.vector.tensor_scalar(
                    cc_sb[:, 2 * k + 1:2 * k + 2], cc_sb[:, 2 * k + 1:2 * k + 2],
                    float(N), None, OP.mult)
            ccr = misc.tile([128, 4], F32, name=f"ccr_b{b}", tag=f"ccr{b}")
            if os.environ.get("KERNEL_ABLATE") == "nocc":
                nc.vector.tensor_scalar(ccr, cc_sb, float(NCORES), None, OP.mult)
            else:
                cc_in = dramp.tile([128, 4], F32, name=f"ccin_b{b}", tag=f"ccin{b}")
                cc_out = dramp.tile([128, 4], F32, name=f"ccout_b{b}", tag=f"ccout{b}",
                                    addr_space="Shared")
                nc.sync.dma_start(out=cc_in, in_=cc_sb)
                nc.gpsimd.collective_compute(
                    "AllReduce", OP.add, replica_groups=rg,
                    ins=[cc_in[:, :]], outs=[cc_out[:, :]])
                nc.sync.dma_start(out=ccr, in_=cc_out)

            # ---- BN affine coefficients ----
            ccr3 = ccr.rearrange("p (k s) -> p k s", k=2)
            mean = misc.tile([128, 2], F32, name=f"mean_b{b}", tag="mean")
            nc.vector.tensor_scalar(mean, ccr3[:, :, 0], 1.0 / BHW, None, OP.mult)
            var = misc.tile([128, 2], F32, name=f"var_b{b}", tag="var")
            # var = sumsq/BHW - mean^2
            m2 = misc.tile([128, 2], F32, name=f"m2_b{b}", tag="m2")
            nc.vector.tensor_mul(m2, mean, mean)
            nc.vector.scalar_tensor_tensor(
                var, ccr3[:, :, 1], 1.0 / BHW, m2, OP.mult, OP.subtract)
            sd = misc.tile([128, 2], F32, name=f"sd_b{b}", tag="sd")
            epst = misc.tile([128, 1], F32, name=f"eps_b{b}", tag="eps")
            nc.vector.memset(epst, EPS)
            nc.scalar.activation(sd, var, AF.Sqrt, bias=epst)
            rs = misc.tile([128, 2], F32, name=f"rs_b{b}", tag="rs")
            nc.vector.reciprocal(rs, sd)
            s2 = misc.tile([128, 2], F32, name=f"s2_b{b}", tag="s2")
            nc.vector.tensor_mul(s2, rs, bp_sb[:, 4:6])
            ms = misc.tile([128, 2], F32, name=f"ms_b{b}", tag="ms")
            nc.vector.tensor_mul(ms, mean, s2)
            t2 = misc.tile([128, 2], F32, name=f"t2_b{b}", tag="t2")
            nc.vector.tensor_sub(t2, bp_sb[:, 6:8], ms)

            # ---- normalize in place and store, chunked for overlap ----
            for k in range(NBLK):
                for jc in range(4):
                    sl = slice(jc * 1024, (jc + 1) * 1024)
                    nc.vector.tensor_scalar(
                        xq[k][:, sl], xq[k][:, sl],
                        s2[:, k:k + 1], t2[:, k:k + 1], OP.mult, OP.add)
                    nc.sync.dma_start(
                        out=out_d[b][k * 128:(k + 1) * 128, sl],
                        in_=xq[k][:, sl])

        branch(0)
        if not ablate.startswith("b0"):
            branch(1)

    nc.finalize()
    return nc


def _get_nc():
    if "nc" not in _CACHE:
        _CACHE["nc"] = _build()
    return _CACHE["nc"]


def _pack_host(inputs):
    import ml_dtypes
    bf16 = ml_dtypes.bfloat16
    f32 = np.float32
    ws = []
    for b in ("1", "2"):
        for w in ("q", "k", "v", "proj"):
            ws.append(np.ascontiguousarray(
                np.asarray(inputs[f"w_{w}{b}"], dtype=f32).T))
    wt = np.concatenate(ws, axis=1).astype(bf16)  # [256, 2048]

    wps = []
    for b in ("1", "2"):
        wpT = np.ascontiguousarray(np.asarray(inputs[f"w_proj{b}"], dtype=f32).T)
        for h in range(NH):
            wps.append(wpT[h * HD:(h + 1) * HD, :])
    wp = np.concatenate(wps, axis=1).astype(bf16)  # [64, 2048]

    bv1 = np.asarray(inputs["b_v1"], dtype=f32)
    bv2 = np.asarray(inputs["b_v2"], dtype=f32)
    g = np.asarray(inputs["gamma"], dtype=f32)
    be = np.asarray(inputs["beta"], dtype=f32)
    bp = np.stack([bv1[:128], bv1[128:], bv2[:128], bv2[128:],
                   g[:128], g[128:], be[:128], be[128:]], axis=1)  # [128, 8]
    bh = np.stack([bv1[h * HD:(h + 1) * HD] for h in range(NH)]
                  + [bv2[h * HD:(h + 1) * HD] for h in range(NH)], axis=1)  # [64, 8]
    return (np.ascontiguousarray(wt), np.ascontiguousarray(wp),
            np.ascontiguousarray(bp), np.ascontiguousarray(bh))


def kernel(rgb_low, rgb_high, dsm_low, dsm_high,
           w_q1, b_q1, w_k1, b_k1, w_v1, b_v1,
           w_q2, b_q2, w_k2, b_k2, w_v2, b_v2,
           w_proj1, b_proj1, w_proj2, b_proj2, gamma, beta,
           _trace=False):
    from concourse.bass_utils import run_bass_kernel_spmd

    inputs = dict(rgb_low=rgb_low, rgb_high=rgb_high, dsm_low=dsm_low,
                  dsm_high=dsm_high, w_q1=w_q1, w_k1=w_k1, w_v1=w_v1,
                  w_proj1=w_proj1, w_q2=w_q2, w_k2=w_k2, w_v2=w_v2,
                  w_proj2=w_proj2, b_v1=b_v1, b_v2=b_v2, gamma=gamma, beta=beta)
    f32 = np.float32
    rl = np.asarray(rgb_low, dtype=f32)
    rh = np.asarray(rgb_high, dtype=f32)
    dl = np.asarray(dsm_low, dtype=f32)
    dh = np.asarray(dsm_high, dtype=f32)
    B = rl.shape[0]
    assert B == NCORES, f"expected batch {NCORES}, got {B}"

    wt, wp, bp, bh = _pack_host(inputs)
    nc = _get_nc()

    in_maps = []
    for i in range(NCORES):
        in_maps.append({
            "xq1": np.ascontiguousarray(rl[i].reshape(C, N)),
            "xkv1": np.ascontiguousarray(dh[i].reshape(C, N)),
            "xq2": np.ascontiguousarray(rh[i].reshape(C, N)),
            "xkv2": np.ascontiguousarray(dl[i].reshape(C, N)),
            "wt": wt, "wp": wp, "bp": bp, "bh": bh,
        })

    res = run_bass_kernel_spmd(nc, in_maps, core_ids=list(range(NCORES)),
                               trace=_trace)
    out_low = np.stack([res.results[i]["out1"].reshape(C, 64, 64)
                        for i in range(NCORES)])
    out_high = np.stack([res.results[i]["out2"].reshape(C, 64, 64)
                         for i in range(NCORES)])
    if _trace:
        _CACHE["last_results"] = res
    return (out_low, out_high, np.asarray(dsm_low), np.asarray(dsm_high))



# revision 16
# speedup vs baseline: 1.2366x; 1.2366x over previous
# Trainium2 Bass kernel for nn_CrossFrequencyInteraction.
#
# Reference computation (per batch item, two symmetric branches):
#   q = Wq @ x_q;  k = Wk @ x_kv;  v = Wv @ x_kv          (1x1 convs, C=256)
#   out = softmax_n(q) used against ctx = softmax_n(k) @ v^T   (linear attention)
#   inter = Wp @ out;  x_q += inter
#   then training-mode BatchNorm over (B,H,W) on both updated rgb tensors.
#
# Sharding: data-parallel over batch (B=8 -> 1 item per core, 8 cores).
# BN statistics (per-channel sum/sumsq) are AllReduced across cores (2KB).
#
# Numerics: all conv-type matmuls run in fp8e4m3 with DoubleRow perf mode
# (256-deep contraction in a single PE pass at 2x rate). Weights are scaled
# x512 on the host to sit in fp8's normal range; the descale is folded into
# the exp() activation scale (exact, power of two). v is rescaled x32 into
# fp8 for the ctx matmul; the 1/32 is folded into the softmax normalizer.
# The residual x rides in bf16; outputs are stored bf16 and upcast on the
# host. Measured end-to-end absmax-rel ~6e-3 (tolerance 2e-2).
#
# Same algebraic restructurings as the bf16 version (all exact): b_q/b_k
# cancel in softmax, b_proj absorbed by BN, softmax normalizers folded into
# the tiny M = Wp . blockdiag(ctx^T) matrix so attention-out + proj become
# one fp8 matmul per branch.

import os
import numpy as np

C = 256
N = 4096
NCORES = 8
BHW = 8 * 64 * 64
EPS = 1e-5
WS = 512.0     # fp8 weight scale
VS = 32.0      # extra fp8 scale on v going into ctx
MS = 256.0     # fp8 scale on mt (the folded Wp.ctx^T matrix)

_CACHE = {}


def _build():
    import concourse.bass as bass
    import concourse.bacc as bacc
    import concourse.tile as tile
    from concourse import mybir
    from contextlib import ExitStack

    F32 = mybir.dt.float32
    BF16 = mybir.dt.bfloat16
    F8 = mybir.dt.float8e4
    OP = mybir.AluOpType
    AF = mybir.ActivationFunctionType
    AX = mybir.AxisListType
    DR = mybir.MatmulPerfMode.DoubleRow

    nc = bacc.Bacc("TRN2", num_devices=NCORES)

    # inputs, packed on host: [c_lo 128, c_blk 2, n 4096]
    xq8_d = [nc.dram_tensor(f"xq8_{b}", [128, 2, N], F8, kind="ExternalInput")
             for b in range(2)]
    xqb_d = [nc.dram_tensor(f"xqb_{b}", [128, 2, N], BF16, kind="ExternalInput")
             for b in range(2)]
    xkv8_d = [nc.dram_tensor(f"xkv8_{b}", [128, 2, N], F8, kind="ExternalInput")
              for b in range(2)]
    # weights: wq [128,2, b*256+co], wkv [128,2, b*512 + (k 0:256 | v 256:512)]
    wq_d = nc.dram_tensor("wq", [128, 2, 512], F8, kind="ExternalInput")
    wkv_d = nc.dram_tensor("wkv", [128, 2, 1024], F8, kind="ExternalInput")
    # wp: [128, (b*2+p)*256 + co] = WpT_b rows p*128:(p+1)*128
    wp_d = nc.dram_tensor("wp", [128, 1024], BF16, kind="ExternalInput")
    # bp: cols 0:2 bv1*32 (blk), 2:4 bv2*32, 4:6 gamma, 6:8 beta
    bp_d = nc.dram_tensor("bp", [128, 8], F32, kind="ExternalInput")
    out_d = [nc.dram_tensor(f"out{b}", [128, 2, N], BF16, kind="ExternalOutput")
             for b in range(2)]

    with ExitStack() as ctx:
        tc = ctx.enter_context(tile.TileContext(nc))
        const = ctx.enter_context(tc.tile_pool(name="const", bufs=1))
        xp = ctx.enter_context(tc.tile_pool(name="xp", bufs=1))
        ekp = ctx.enter_context(tc.tile_pool(name="ekp", bufs=4))
        eqp = ctx.enter_context(tc.tile_pool(name="eqp", bufs=1))
        misc = ctx.enter_context(tc.tile_pool(name="misc", bufs=1))
        scrp = ctx.enter_context(tc.tile_pool(name="scrp", bufs=2))
        outp = ctx.enter_context(tc.tile_pool(name="outp", bufs=4))
        pkvp = ctx.enter_context(tc.tile_pool(name="pkvp", bufs=3, space="PSUM"))
        qip = ctx.enter_context(tc.tile_pool(name="qip", bufs=2, space="PSUM"))
        ctxp = ctx.enter_context(tc.tile_pool(name="ctxp", bufs=1, space="PSUM"))
        pmtp = ctx.enter_context(tc.tile_pool(name="pmtp", bufs=1, space="PSUM"))
        dramp = ctx.enter_context(tc.tile_pool(name="dramp", bufs=1, space="DRAM"))

        rg = [list(range(NCORES))]

        # ---- constants (sync queue, ahead of inputs) ----
        wkv_sb = const.tile([128, 2, 1024], F8, name="wkv", tag="wkv")
        nc.sync.dma_start(out=wkv_sb, in_=wkv_d[:, :, :])
        wq_sb = const.tile([128, 2, 512], F8, name="wq", tag="wq")
        nc.sync.dma_start(out=wq_sb, in_=wq_d[:, :, :])
        # ---- inputs, issued upfront in consumption order ----
        xkv8 = []
        xq8 = []
        xqb = []
        for b in range(2):
            xkv8.append(xp.tile([128, 2, N], F8, name=f"xkv8_{b}", tag=f"xkv8_{b}"))
            xq8.append(xp.tile([128, 2, N], F8, name=f"xq8_{b}", tag=f"xq8_{b}"))
            xqb.append(xp.tile([128, 2, N], BF16, name=f"xqb_{b}", tag=f"xqb_{b}"))
        for h in range(2):
            sl = slice(h * 2048, (h + 1) * 2048)
            nc.sync.dma_start(out=xkv8[0][:, :, sl], in_=xkv8_d[0][:, :, sl])
        for h in range(2):
            sl = slice(h * 2048, (h + 1) * 2048)
            nc.sync.dma_start(out=xq8[0][:, :, sl], in_=xq8_d[0][:, :, sl])
        wp_sb = const.tile([128, 1024], BF16, name="wp", tag="wp")
        nc.sync.dma_start(out=wp_sb, in_=wp_d[:, :])
        bp_sb = const.tile([128, 8], F32, name="bp", tag="bp")
        nc.sync.dma_start(out=bp_sb, in_=bp_d[:, :])
        for h in range(2):
            sl = slice(h * 2048, (h + 1) * 2048)
            nc.sync.dma_start(out=xqb[0][:, :, sl], in_=xqb_d[0][:, :, sl])
        for b in (1,):
            for h in range(2):
                sl = slice(h * 2048, (h + 1) * 2048)
                nc.sync.dma_start(out=xkv8[b][:, :, sl], in_=xkv8_d[b][:, :, sl])
            for h in range(2):
                sl = slice(h * 2048, (h + 1) * 2048)
                nc.sync.dma_start(out=xq8[b][:, :, sl], in_=xq8_d[b][:, :, sl])
            for h in range(2):
                sl = slice(h * 2048, (h + 1) * 2048)
                nc.sync.dma_start(out=xqb[b][:, :, sl], in_=xqb_d[b][:, :, sl])

        ones8 = const.tile([128, 2, 128], F8, name="ones8", tag="ones8")
        nc.vector.memset(ones8, 1.0)
        epst = const.tile([128, 1], F32, name="epst", tag="epst")
        nc.vector.memset(epst, EPS)

        # per-branch state carried from compute to finish
        st = [{}, {}]

        def compute(b):
            wofs = b * 512
            # ---- KV phase: one DR matmul pass per 128-n tile ----
            pctxf = ctxp.tile([128, 512], F32, name=f"pctx_{b}", tag="pctx")
            pctx = pctxf[:, 0:256]
            pden = pctxf[:, 256:512]
            prev = None
            for p in range(16):
                ek2 = ekp.tile([128, 2, 256], F8, name=f"ek_{b}_{p}", tag="ek")
                vt2 = ekp.tile([128, 2, 256], F8, name=f"vt_{b}_{p}", tag="vt")
                for tt in range(2):
                    t = 2 * p + tt
                    pkv = pkvp.tile([128, 512], F32, name=f"pkv_{b}_{t}", tag="pkv")
                    nc.tensor.matmul(
                        pkv[:, 0:256],
                        lhsT=xkv8[b][:, :, t * 128:(t + 1) * 128],
                        rhs=wkv_sb[:, :, wofs:wofs + 256],
                        start=True, stop=True, perf_mode=DR)
                    nc.tensor.matmul(
                        pkv[:, 256:512],
                        lhsT=xkv8[b][:, :, t * 128:(t + 1) * 128],
                        rhs=wkv_sb[:, :, wofs + 256:wofs + 512],
                        start=True, stop=True, perf_mode=DR)
                    nc.scalar.activation(ek2[:, tt, :], pkv[:, 0:256], AF.Exp,
                                         scale=1.0 / WS)
                    nc.vector.tensor_scalar(vt2[:, tt, :], pkv[:, 256:512],
                                            VS / WS, None, OP.mult)
                if prev is not None:
                    pe, pv, pp = prev
                    for blk in range(2):
                        nc.tensor.matmul(
                            pctx[:, blk * 128:(blk + 1) * 128],
                            lhsT=pv[:, :, blk * 128:(blk + 1) * 128],
                            rhs=pe[:, :, blk * 128:(blk + 1) * 128],
                            start=(pp == 0), stop=False, perf_mode=DR,
                            skip_group_check=True)
                    nc.tensor.matmul(
                        pden, lhsT=ones8, rhs=pe[:, :, :],
                        start=(pp == 0), stop=False, perf_mode=DR,
                        skip_group_check=True)
                prev = (ek2, vt2, p)
            pe, pv, pp = prev
            for blk in range(2):
                nc.tensor.matmul(
                    pctx[:, blk * 128:(blk + 1) * 128],
                    lhsT=pv[:, :, blk * 128:(blk + 1) * 128],
                    rhs=pe[:, :, blk * 128:(blk + 1) * 128],
                    start=False, stop=True, perf_mode=DR,
                    skip_group_check=True)
            nc.tensor.matmul(
                pden, lhsT=ones8, rhs=pe[:, :, :],
                start=False, stop=True, perf_mode=DR,
                skip_group_check=True)

            # ---- ctx evict (+ scaled b_v fold) and k-softmax denominators ----
            ctx2 = misc.tile([128, 256], BF16, name=f"ctx2_{b}", tag=f"ctx2_{b}")
            for blk in range(2):
                nc.vector.tensor_scalar(
                    ctx2[:, blk * 128:(blk + 1) * 128],
                    pctx[:, blk * 128:(blk + 1) * 128],
                    bp_sb[:, 2 * b + blk:2 * b + blk + 1], None, OP.add)
            drow = misc.tile([1, 256], F32, name=f"drow_{b}", tag="drow")
            nc.vector.tensor_copy(drow, pden[0:1, :])
            denT = misc.tile([128, 2], F32, name=f"denT_{b}", tag="denT")
            for blk in range(2):
                nc.sync.dma_start(out=denT[:, blk:blk + 1],
                                  in_=drow[0:1, blk * 128:(blk + 1) * 128])

            # ---- Q phase: DR conv + exp with accumulated row sums ----
            expq8 = eqp.tile([128, 2, N], F8, name=f"expq_{b}", tag=f"expq_{b}")
            sqp = misc.tile([128, 2, 16], F32, name=f"sqp_{b}", tag="sqp")
            for k in range(2):
                for jp in range(8):
                    pq2 = qip.tile([128, 512], F32, name=f"pq_{b}_{k}_{jp}",
                                   tag="mm256")
                    for tj in range(2):
                        j = 2 * jp + tj
                        nc.tensor.matmul(
                            pq2[:, tj * 256:(tj + 1) * 256],
                            lhsT=wq_sb[:, :,
                                       b * 256 + k * 128:b * 256 + (k + 1) * 128],
                            rhs=xq8[b][:, :, j * 256:(j + 1) * 256],
                            start=True, stop=True, perf_mode=DR)
                        nc.scalar.activation(
                            expq8[:, k, j * 256:(j + 1) * 256],
                            pq2[:, tj * 256:(tj + 1) * 256], AF.Exp,
                            scale=1.0 / WS, accum_out=sqp[:, k, j:j + 1])

            # ---- normalization: fac = 8 / (den_k * sum_q)  (8 = 256/32) ----
            sq2 = misc.tile([128, 2], F32, name=f"sq2_{b}", tag="sq2")
            for k in range(2):
                nc.vector.reduce_sum(sq2[:, k:k + 1], sqp[:, k, :], axis=AX.X)
            fde = misc.tile([128, 2], F32, name=f"fde_{b}", tag="fde")
            nc.vector.tensor_scalar(fde, sq2, MS / VS, None, OP.mult)
            nc.vector.tensor_mul(fde, fde, denT)
            fac = misc.tile([128, 2], F32, name=f"fac_{b}", tag="fac")
            nc.vector.reciprocal(fac, fde)

            # ---- M^T: per-head Wp . ctx^T with folded normalization ----
            mt8 = misc.tile([128, 2, 256], F8, name=f"mt_{b}", tag=f"mt_{b}")
            pmtf = pmtp.tile([128, 512], F32, name=f"pmt_{b}", tag="pmt")
            for p in range(2):
                pmt = pmtf[:, p * 256:(p + 1) * 256]
                for hh in range(2):
                    h = 2 * p + hh
                    nc.tensor.matmul(
                        pmt[hh * 64:(hh + 1) * 64, :],
                        lhsT=ctx2[hh * 64:(hh + 1) * 64, h * 64:(h + 1) * 64],
                        rhs=wp_sb[hh * 64:(hh + 1) * 64,
                                  (b * 2 + p) * 256:(b * 2 + p + 1) * 256],
                        start=True, stop=True)
                nc.vector.tensor_scalar(mt8[:, p, :], pmt, fac[:, p:p + 1],
                                        None, OP.mult)

            # ---- inter = M^T.T @ expq (fp8 DR), residual + stats on gpsimd ----
            sums = misc.tile([128, 2, 16, 2], F32, name=f"sums_{b}", tag="sums")
            for k in range(2):
                for jp in range(8):
                    pi2 = qip.tile([128, 512], F32, name=f"pi_{b}_{k}_{jp}",
                                   tag="mm256")
                    for tj in range(2):
                        j = 2 * jp + tj
                        nc.tensor.matmul(
                            pi2[:, tj * 256:(tj + 1) * 256],
                            lhsT=mt8[:, :, k * 128:(k + 1) * 128],
                            rhs=expq8[:, :, j * 256:(j + 1) * 256],
                            start=True, stop=True, perf_mode=DR)
                        xs = xqb[b][:, k, j * 256:(j + 1) * 256]
                        nc.vector.scalar_tensor_tensor(
                            xs, pi2[:, tj * 256:(tj + 1) * 256], 1.0 / MS, xs,
                            OP.mult, OP.add, accum_out=sums[:, k, j, 0:1])
                        scr = scrp.tile([128, 256], F32,
                                        name=f"scr_{b}_{k}_{j}", tag="scr")
                        nc.gpsimd.tensor_tensor(scr, xs, xs, OP.mult)
                        nc.vector.reduce_sum(sums[:, k, j, 1:2], scr, axis=AX.X)

            # ---- pack per-core sums and AllReduce ----
            cc_sb = misc.tile([128, 4], F32, name=f"cc_{b}", tag=f"cc{b}")
            for k in range(2):
                for s in range(2):
                    nc.vector.reduce_sum(cc_sb[:, 2 * k + s:2 * k + s + 1],
                                         sums[:, k, :, s], axis=AX.X)
            cc_in = dramp.tile([128, 4], F32, name=f"ccin_{b}", tag=f"ccin{b}")
            cc_out = dramp.tile([128, 4], F32, name=f"ccout_{b}", tag=f"ccout{b}",
                                addr_space="Shared")
            nc.sync.dma_start(out=cc_in, in_=cc_sb)
            if os.environ.get("KERNEL_ABLATE") == "nocc":
                ccr = misc.tile([128, 4], F32, name=f"ccr_{b}", tag=f"ccr{b}")
                nc.vector.tensor_scalar(ccr, cc_sb, float(NCORES), None, OP.mult)
                st[b]["ccr"] = ccr
            else:
                nc.gpsimd.collective_compute(
                    "AllReduce", OP.add, replica_groups=rg,
                    ins=[cc_in[:, :]], outs=[cc_out[:, :]])
                st[b]["cc_out"] = cc_out

        def finish(b):
            if "cc_out" in st[b]:
                ccr = misc.tile([128, 4], F32, name=f"ccr_{b}", tag=f"ccr{b}")
                nc.gpsimd.dma_start(out=ccr, in_=st[b]["cc_out"])
            else:
                ccr = st[b]["ccr"]
            ccr3 = ccr.rearrange("p (k s) -> p k s", k=2)
            mean = misc.tile([128, 2], F32, name=f"mean_{b}", tag="mean")
            nc.vector.tensor_scalar(mean, ccr3[:, :, 0], 1.0 / BHW, None, OP.mult)
            m2 = misc.tile([128, 2], F32, name=f"m2_{b}", tag="m2")
            nc.vector.tensor_mul(m2, mean, mean)
            var = misc.tile([128, 2], F32, name=f"var_{b}", tag="var")
            nc.vector.scalar_tensor_tensor(
                var, ccr3[:, :, 1], 1.0 / BHW, m2, OP.mult, OP.subtract)
            sd = misc.tile([128, 2], F32, name=f"sd_{b}", tag="sd")
            nc.scalar.activation(sd, var, AF.Sqrt, bias=epst)
            rs = misc.tile([128, 2], F32, name=f"rs_{b}", tag="rs")
            nc.vector.reciprocal(rs, sd)
            s2 = misc.tile([128, 2], F32, name=f"s2_{b}", tag="s2")
            nc.vector.tensor_mul(s2, rs, bp_sb[:, 4:6])
            ms = misc.tile([128, 2], F32, name=f"ms_{b}", tag="ms")
            nc.vector.tensor_mul(ms, mean, s2)
            t2 = misc.tile([128, 2], F32, name=f"t2_{b}", tag="t2")
            nc.vector.tensor_sub(t2, bp_sb[:, 6:8], ms)

            for k in range(2):
                for c in range(4):
                    sl = slice(c * 1024, (c + 1) * 1024)
                    ob = outp.tile([128, 1024], BF16, name=f"ob_{b}_{k}_{c}",
                                   tag="ob")
                    nc.vector.tensor_scalar(
                        ob, xqb[b][:, k, sl],
                        s2[:, k:k + 1], t2[:, k:k + 1], OP.mult, OP.add)
                    nc.sync.dma_start(out=out_d[b][:, k, sl], in_=ob)

        compute(0)
        compute(1)
        finish(0)
        finish(1)

    nc.finalize()
    return nc


def _get_nc():
    if "nc" not in _CACHE:
        _CACHE["nc"] = _build()
    return _CACHE["nc"]


def _pack_host(inputs):
    import ml_dtypes
    bf16 = ml_dtypes.bfloat16
    f8 = ml_dtypes.float8_e4m3
    f32 = np.float32

    def blkfold(a):  # [256, cols] -> [128, 2, cols]
        return np.ascontiguousarray(
            a.reshape(2, 128, -1).transpose(1, 0, 2))

    wqs = []
    wkvs = []
    wps = []
    for b in ("1", "2"):
        wqs.append(blkfold(np.asarray(inputs[f"w_q{b}"], f32).T * WS))
        wkvs.append(blkfold(np.concatenate(
            [np.asarray(inputs[f"w_k{b}"], f32).T,
             np.asarray(inputs[f"w_v{b}"], f32).T], axis=1) * WS))
        wpT = np.ascontiguousarray(np.asarray(inputs[f"w_proj{b}"], f32).T)
        wps.append(wpT[0:128, :])
        wps.append(wpT[128:256, :])
    wq = np.ascontiguousarray(np.concatenate(wqs, axis=2)).astype(f8)
    wkv = np.ascontiguousarray(np.concatenate(wkvs, axis=2)).astype(f8)
    wp = np.ascontiguousarray(np.concatenate(wps, axis=1)).astype(bf16)

    bv1 = np.asarray(inputs["b_v1"], f32) * VS
    bv2 = np.asarray(inputs["b_v2"], f32) * VS
    g = np.asarray(inputs["gamma"], f32)
    be = np.asarray(inputs["beta"], f32)
    bp = np.stack([bv1[:128], bv1[128:], bv2[:128], bv2[128:],
                   g[:128], g[128:], be[:128], be[128:]], axis=1)
    return wq, wkv, wp, np.ascontiguousarray(bp)


def kernel(rgb_low, rgb_high, dsm_low, dsm_high,
           w_q1, b_q1, w_k1, b_k1, w_v1, b_v1,
           w_q2, b_q2, w_k2, b_k2, w_v2, b_v2,
           w_proj1, b_proj1, w_proj2, b_proj2, gamma, beta,
           _trace=False):
    import ml_dtypes
    from concourse.bass_utils import run_bass_kernel_spmd
    bf16 = ml_dtypes.bfloat16
    f8 = ml_dtypes.float8_e4m3
    f32 = np.float32

    inputs = dict(w_q1=w_q1, w_k1=w_k1, w_v1=w_v1, w_proj1=w_proj1,
                  w_q2=w_q2, w_k2=w_k2, w_v2=w_v2, w_proj2=w_proj2,
                  b_v1=b_v1, b_v2=b_v2, gamma=gamma, beta=beta)
    wq, wkv, wp, bp = _pack_host(inputs)
    nc = _get_nc()

    def blkfold(a):  # [256, N] -> [128, 2, N]
        return np.ascontiguousarray(a.reshape(2, 128, N).transpose(1, 0, 2))

    srcs = [(np.asarray(rgb_low, f32), np.asarray(dsm_high, f32)),
            (np.asarray(rgb_high, f32), np.asarray(dsm_low, f32))]
    B = srcs[0][0].shape[0]
    assert B == NCORES, f"expected batch {NCORES}, got {B}"

    in_maps = []
    for i in range(NCORES):
        m = {"wq": wq, "wkv": wkv, "wp": wp, "bp": bp}
        for b in range(2):
            xq = blkfold(srcs[b][0][i].reshape(C, N))
            xkv = blkfold(srcs[b][1][i].reshape(C, N))
            m[f"xq8_{b}"] = xq.astype(f8)
            m[f"xqb_{b}"] = xq.astype(bf16)
            m[f"xkv8_{b}"] = xkv.astype(f8)
        in_maps.append(m)

    res = run_bass_kernel_spmd(nc, in_maps, core_ids=list(range(NCORES)),
                               trace=_trace)

    outs = []
    for b in range(2):
        o = np.stack([
            np.asarray(res.results[i][f"out{b}"], f32)
            .transpose(1, 0, 2).reshape(C, 64, 64)
            for i in range(NCORES)])
        outs.append(o)
    if _trace:
        _CACHE["last_results"] = res
    return (outs[0], outs[1], np.asarray(dsm_low), np.asarray(dsm_high))


# revision 20
# speedup vs baseline: 1.3259x; 1.0723x over previous
# Trainium2 Bass kernel for nn_CrossFrequencyInteraction.
#
# Reference computation (per batch item, two symmetric branches):
#   q = Wq @ x_q;  k = Wk @ x_kv;  v = Wv @ x_kv          (1x1 convs, C=256)
#   out = softmax_n(q) used against ctx = softmax_n(k) @ v^T   (linear attention)
#   inter = Wp @ out;  x_q += inter
#   then training-mode BatchNorm over (B,H,W) on both updated rgb tensors.
#
# Sharding: data-parallel over batch (B=8 -> 1 item per core, 8 cores).
# BN statistics (per-channel sum/sumsq) are AllReduced across cores (2KB).
#
# Numerics: all conv-type matmuls run in fp8e4m3 with DoubleRow perf mode
# (256-deep contraction in a single PE pass at 2x rate). Weights are scaled
# x512 on the host to sit in fp8's normal range; the descale is folded into
# the exp() activation scale (exact, power of two). v is rescaled x32 into
# fp8 for the ctx matmul; the 1/32 is folded into the softmax normalizer.
# The residual x rides in bf16; outputs are stored bf16 and upcast on the
# host. Measured end-to-end absmax-rel ~6e-3 (tolerance 2e-2).
#
# Same algebraic restructurings as the bf16 version (all exact): b_q/b_k
# cancel in softmax, b_proj absorbed by BN, softmax normalizers folded into
# the tiny M = Wp . blockdiag(ctx^T) matrix so attention-out + proj become
# one fp8 matmul per branch.

import os
import numpy as np

C = 256
N = 4096
NCORES = 8
BHW = 8 * 64 * 64
EPS = 1e-5
WS = 512.0     # fp8 weight scale
VS = 32.0      # extra fp8 scale on v going into ctx
MS = 256.0     # fp8 scale on mt (the folded Wp.ctx^T matrix)

_CACHE = {}


def _build():
    import concourse.bass as bass
    import concourse.bacc as bacc
    import concourse.tile as tile
    from concourse import mybir
    from contextlib import ExitStack

    F32 = mybir.dt.float32
    BF16 = mybir.dt.bfloat16
    F8 = mybir.dt.float8e4
    OP = mybir.AluOpType
    AF = mybir.ActivationFunctionType
    AX = mybir.AxisListType
    DR = mybir.MatmulPerfMode.DoubleRow

    nc = bacc.Bacc("TRN2", num_devices=NCORES)

    # inputs, packed on host: [c_lo 128, c_blk 2, n 4096]
    xq8_d = [nc.dram_tensor(f"xq8_{b}", [128, 2, N], F8, kind="ExternalInput")
             for b in range(2)]
    xqb_d = [nc.dram_tensor(f"xqb_{b}", [128, 2, N], BF16, kind="ExternalInput")
             for b in range(2)]
    xkv8_d = [nc.dram_tensor(f"xkv8_{b}", [128, 2, N], F8, kind="ExternalInput")
              for b in range(2)]
    # weights: wq [128,2, b*256+co], wkv [128,2, b*512 + (k 0:256 | v 256:512)]
    wq_d = nc.dram_tensor("wq", [128, 2, 512], F8, kind="ExternalInput")
    wkv_d = nc.dram_tensor("wkv", [128, 2, 1024], F8, kind="ExternalInput")
    # wp: [128, (b*2+p)*256 + co] = WpT_b rows p*128:(p+1)*128
    wp_d = nc.dram_tensor("wp", [128, 1024], BF16, kind="ExternalInput")
    # bp: cols 0:2 bv1*32 (blk), 2:4 bv2*32, 4:6 gamma, 6:8 beta
    bp_d = nc.dram_tensor("bp", [128, 8], F32, kind="ExternalInput")
    out_d = [nc.dram_tensor(f"out{b}", [128, 2, N], BF16, kind="ExternalOutput")
             for b in range(2)]

    with ExitStack() as ctx:
        tc = ctx.enter_context(tile.TileContext(nc))
        const = ctx.enter_context(tc.tile_pool(name="const", bufs=1))
        xp = ctx.enter_context(tc.tile_pool(name="xp", bufs=1))
        ekp = ctx.enter_context(tc.tile_pool(name="ekp", bufs=4))
        eqp = ctx.enter_context(tc.tile_pool(name="eqp", bufs=1))
        misc = ctx.enter_context(tc.tile_pool(name="misc", bufs=1))
        scrp = ctx.enter_context(tc.tile_pool(name="scrp", bufs=2))
        outp = ctx.enter_context(tc.tile_pool(name="outp", bufs=4))
        pkvp = ctx.enter_context(tc.tile_pool(name="pkvp", bufs=3, space="PSUM"))
        qip = ctx.enter_context(tc.tile_pool(name="qip", bufs=2, space="PSUM"))
        ctxp = ctx.enter_context(tc.tile_pool(name="ctxp", bufs=1, space="PSUM"))
        pmtp = ctx.enter_context(tc.tile_pool(name="pmtp", bufs=1, space="PSUM"))
        dramp = ctx.enter_context(tc.tile_pool(name="dramp", bufs=1, space="DRAM"))

        rg = [list(range(NCORES))]

        # ---- constants (sync queue, ahead of inputs) ----
        wkv_sb = const.tile([128, 2, 1024], F8, name="wkv", tag="wkv")
        nc.sync.dma_start(out=wkv_sb, in_=wkv_d[:, :, :])
        wq_sb = const.tile([128, 2, 512], F8, name="wq", tag="wq")
        nc.sync.dma_start(out=wq_sb, in_=wq_d[:, :, :])
        # ---- inputs, issued upfront in consumption order ----
        xkv8 = []
        xq8 = []
        xqb = []
        for b in range(2):
            xkv8.append(xp.tile([128, 2, N], F8, name=f"xkv8_{b}", tag=f"xkv8_{b}"))
            xq8.append(xp.tile([128, 2, N], F8, name=f"xq8_{b}", tag=f"xq8_{b}"))
            xqb.append(xp.tile([128, 2, N], BF16, name=f"xqb_{b}", tag=f"xqb_{b}"))
        for h in range(2):
            sl = slice(h * 2048, (h + 1) * 2048)
            nc.sync.dma_start(out=xkv8[0][:, :, sl], in_=xkv8_d[0][:, :, sl])
        for h in range(2):
            sl = slice(h * 2048, (h + 1) * 2048)
            nc.sync.dma_start(out=xq8[0][:, :, sl], in_=xq8_d[0][:, :, sl])
        wp_sb = const.tile([128, 1024], BF16, name="wp", tag="wp")
        nc.sync.dma_start(out=wp_sb, in_=wp_d[:, :])
        bp_sb = const.tile([128, 8], F32, name="bp", tag="bp")
        nc.sync.dma_start(out=bp_sb, in_=bp_d[:, :])
        for h in range(2):
            sl = slice(h * 2048, (h + 1) * 2048)
            nc.sync.dma_start(out=xqb[0][:, :, sl], in_=xqb_d[0][:, :, sl])
        for b in (1,):
            for h in range(2):
                sl = slice(h * 2048, (h + 1) * 2048)
                nc.sync.dma_start(out=xkv8[b][:, :, sl], in_=xkv8_d[b][:, :, sl])
            for h in range(2):
                sl = slice(h * 2048, (h + 1) * 2048)
                nc.sync.dma_start(out=xq8[b][:, :, sl], in_=xq8_d[b][:, :, sl])
            for h in range(2):
                sl = slice(h * 2048, (h + 1) * 2048)
                nc.sync.dma_start(out=xqb[b][:, :, sl], in_=xqb_d[b][:, :, sl])

        ones8 = const.tile([128, 2, 128], F8, name="ones8", tag="ones8")
        nc.vector.memset(ones8, 1.0)
        epst = const.tile([128, 1], F32, name="epst", tag="epst")
        nc.vector.memset(epst, EPS)

        # per-branch state carried from compute to finish
        st = [{}, {}]

        def compute(b):
            wofs = b * 512
            # ---- KV phase: one DR matmul pass per 128-n tile ----
            pctxf = ctxp.tile([128, 512], F32, name=f"pctx_{b}", tag="pctx")
            pctx = pctxf[:, 0:256]
            pden = pctxf[:, 256:512]
            prev = None
            for p in range(16):
                ek2 = ekp.tile([128, 2, 256], F8, name=f"ek_{b}_{p}", tag="ek")
                vt2 = ekp.tile([128, 2, 256], F8, name=f"vt_{b}_{p}", tag="vt")
                for tt in range(2):
                    t = 2 * p + tt
                    pkv = pkvp.tile([128, 512], F32, name=f"pkv_{b}_{t}", tag="pkv")
                    nc.tensor.matmul(
                        pkv[:, 0:256],
                        lhsT=xkv8[b][:, :, t * 128:(t + 1) * 128],
                        rhs=wkv_sb[:, :, wofs:wofs + 256],
                        start=True, stop=True, perf_mode=DR)
                    nc.tensor.matmul(
                        pkv[:, 256:512],
                        lhsT=xkv8[b][:, :, t * 128:(t + 1) * 128],
                        rhs=wkv_sb[:, :, wofs + 256:wofs + 512],
                        start=True, stop=True, perf_mode=DR)
                    nc.scalar.activation(ek2[:, tt, :], pkv[:, 0:256], AF.Exp,
                                         scale=1.0 / WS)
                    nc.vector.tensor_scalar(vt2[:, tt, :], pkv[:, 256:512],
                                            VS / WS, None, OP.mult)
                if prev is not None:
                    pe, pv, pp = prev
                    for blk in range(2):
                        nc.tensor.matmul(
                            pctx[:, blk * 128:(blk + 1) * 128],
                            lhsT=pv[:, :, blk * 128:(blk + 1) * 128],
                            rhs=pe[:, :, blk * 128:(blk + 1) * 128],
                            start=(pp == 0), stop=False, perf_mode=DR,
                            skip_group_check=True)
                    nc.tensor.matmul(
                        pden, lhsT=ones8, rhs=pe[:, :, :],
                        start=(pp == 0), stop=False, perf_mode=DR,
                        skip_group_check=True)
                prev = (ek2, vt2, p)
            pe, pv, pp = prev
            for blk in range(2):
                nc.tensor.matmul(
                    pctx[:, blk * 128:(blk + 1) * 128],
                    lhsT=pv[:, :, blk * 128:(blk + 1) * 128],
                    rhs=pe[:, :, blk * 128:(blk + 1) * 128],
                    start=False, stop=True, perf_mode=DR,
                    skip_group_check=True)
            nc.tensor.matmul(
                pden, lhsT=ones8, rhs=pe[:, :, :],
                start=False, stop=True, perf_mode=DR,
                skip_group_check=True)

            # ---- ctx evict (+ scaled b_v fold) and k-softmax denominators ----
            ctx2 = misc.tile([128, 256], BF16, name=f"ctx2_{b}", tag=f"ctx2_{b}")
            for blk in range(2):
                nc.vector.tensor_scalar(
                    ctx2[:, blk * 128:(blk + 1) * 128],
                    pctx[:, blk * 128:(blk + 1) * 128],
                    bp_sb[:, 2 * b + blk:2 * b + blk + 1], None, OP.add)
            drow = misc.tile([1, 256], F32, name=f"drow_{b}", tag="drow")
            nc.vector.tensor_copy(drow, pden[0:1, :])
            denT = misc.tile([128, 2], F32, name=f"denT_{b}", tag="denT")
            for blk in range(2):
                nc.sync.dma_start(out=denT[:, blk:blk + 1],
                                  in_=drow[0:1, blk * 128:(blk + 1) * 128])

            # ---- Q phase: DR conv + exp with accumulated row sums ----
            expq8 = eqp.tile([128, 2, N], F8, name=f"expq_{b}", tag=f"expq_{b}")
            sqp = misc.tile([128, 2, 8], F32, name=f"sqp_{b}", tag="sqp")
            for k in range(2):
                for jp in range(8):
                    pq2 = qip.tile([128, 512], F32, name=f"pq_{b}_{k}_{jp}",
                                   tag="mm256")
                    for tj in range(2):
                        j = 2 * jp + tj
                        nc.tensor.matmul(
                            pq2[:, tj * 256:(tj + 1) * 256],
                            lhsT=wq_sb[:, :,
                                       b * 256 + k * 128:b * 256 + (k + 1) * 128],
                            rhs=xq8[b][:, :, j * 256:(j + 1) * 256],
                            start=True, stop=True, perf_mode=DR)
                    nc.scalar.activation(
                        expq8[:, k, jp * 512:(jp + 1) * 512],
                        pq2, AF.Exp,
                        scale=1.0 / WS, accum_out=sqp[:, k, jp:jp + 1])

            # ---- normalization: fac = 8 / (den_k * sum_q)  (8 = 256/32) ----
            sq2 = misc.tile([128, 2], F32, name=f"sq2_{b}", tag="sq2")
            for k in range(2):
                nc.vector.reduce_sum(sq2[:, k:k + 1], sqp[:, k, :], axis=AX.X)
            fde = misc.tile([128, 2], F32, name=f"fde_{b}", tag="fde")
            nc.vector.tensor_scalar(fde, sq2, MS / VS, None, OP.mult)
            nc.vector.tensor_mul(fde, fde, denT)
            fac = misc.tile([128, 2], F32, name=f"fac_{b}", tag="fac")
            nc.vector.reciprocal(fac, fde)

            # ---- M^T: per-head Wp . ctx^T with folded normalization ----
            mt8 = misc.tile([128, 2, 256], F8, name=f"mt_{b}", tag=f"mt_{b}")
            pmtf = pmtp.tile([128, 512], F32, name=f"pmt_{b}", tag="pmt")
            for p in range(2):
                pmt = pmtf[:, p * 256:(p + 1) * 256]
                for hh in range(2):
                    h = 2 * p + hh
                    nc.tensor.matmul(
                        pmt[hh * 64:(hh + 1) * 64, :],
                        lhsT=ctx2[hh * 64:(hh + 1) * 64, h * 64:(h + 1) * 64],
                        rhs=wp_sb[hh * 64:(hh + 1) * 64,
                                  (b * 2 + p) * 256:(b * 2 + p + 1) * 256],
                        start=True, stop=True)
                nc.vector.tensor_scalar(mt8[:, p, :], pmt, fac[:, p:p + 1],
                                        None, OP.mult)

            # ---- inter = M^T.T @ expq (fp8 DR), residual + stats on gpsimd ----
            sumr = misc.tile([128, 2, 8], F32, name=f"sumr_{b}", tag="sumr")
            ssq = misc.tile([128, 2, 4], F32, name=f"ssq_{b}", tag="ssq")
            for k in range(2):
                for jp in range(8):
                    pi2 = qip.tile([128, 512], F32, name=f"pi_{b}_{k}_{jp}",
                                   tag="mm256")
                    for tj in range(2):
                        j = 2 * jp + tj
                        nc.tensor.matmul(
                            pi2[:, tj * 256:(tj + 1) * 256],
                            lhsT=mt8[:, :, k * 128:(k + 1) * 128],
                            rhs=expq8[:, :, j * 256:(j + 1) * 256],
                            start=True, stop=True, perf_mode=DR)
                    xs2 = xqb[b][:, k, jp * 512:(jp + 1) * 512]
                    nc.vector.scalar_tensor_tensor(
                        xs2, pi2, 1.0 / MS, xs2,
                        OP.mult, OP.add, accum_out=sumr[:, k, jp:jp + 1])
                    if jp % 2 == 1:
                        c = jp // 2
                        xc = xqb[b][:, k, c * 1024:(c + 1) * 1024]
                        scr = scrp.tile([128, 1024], F32,
                                        name=f"scr_{b}_{k}_{c}", tag="scr")
                        nc.gpsimd.tensor_tensor(scr, xc, xc, OP.mult)
                        nc.vector.reduce_sum(ssq[:, k, c:c + 1], scr, axis=AX.X)

            # ---- pack per-core sums and AllReduce ----
            cc_sb = misc.tile([128, 4], F32, name=f"cc_{b}", tag=f"cc{b}")
            for k in range(2):
                nc.vector.reduce_sum(cc_sb[:, 2 * k:2 * k + 1],
                                     sumr[:, k, :], axis=AX.X)
                nc.vector.reduce_sum(cc_sb[:, 2 * k + 1:2 * k + 2],
                                     ssq[:, k, :], axis=AX.X)
            cc_in = dramp.tile([128, 4], F32, name=f"ccin_{b}", tag=f"ccin{b}")
            cc_out = dramp.tile([128, 4], F32, name=f"ccout_{b}", tag=f"ccout{b}",
                                addr_space="Shared")
            nc.sync.dma_start(out=cc_in, in_=cc_sb)
            if os.environ.get("KERNEL_ABLATE") == "nocc":
                ccr = misc.tile([128, 4], F32, name=f"ccr_{b}", tag=f"ccr{b}")
                nc.vector.tensor_scalar(ccr, cc_sb, float(NCORES), None, OP.mult)
                st[b]["ccr"] = ccr
            else:
                nc.gpsimd.collective_compute(
                    "AllReduce", OP.add, replica_groups=rg,
                    ins=[cc_in[:, :]], outs=[cc_out[:, :]])
                st[b]["cc_out"] = cc_out

        def finish(b):
            if "cc_out" in st[b]:
                ccr = misc.tile([128, 4], F32, name=f"ccr_{b}", tag=f"ccr{b}")
                nc.gpsimd.dma_start(out=ccr, in_=st[b]["cc_out"])
            else:
                ccr = st[b]["ccr"]
            ccr3 = ccr.rearrange("p (k s) -> p k s", k=2)
            mean = misc.tile([128, 2], F32, name=f"mean_{b}", tag="mean")
            nc.vector.tensor_scalar(mean, ccr3[:, :, 0], 1.0 / BHW, None, OP.mult)
            m2 = misc.tile([128, 2], F32, name=f"m2_{b}", tag="m2")
            nc.vector.tensor_mul(m2, mean, mean)
            var = misc.tile([128, 2], F32, name=f"var_{b}", tag="var")
            nc.vector.scalar_tensor_tensor(
                var, ccr3[:, :, 1], 1.0 / BHW, m2, OP.mult, OP.subtract)
            sd = misc.tile([128, 2], F32, name=f"sd_{b}", tag="sd")
            nc.scalar.activation(sd, var, AF.Sqrt, bias=epst)
            rs = misc.tile([128, 2], F32, name=f"rs_{b}", tag="rs")
            nc.vector.reciprocal(rs, sd)
            s2 = misc.tile([128, 2], F32, name=f"s2_{b}", tag="s2")
            nc.vector.tensor_mul(s2, rs, bp_sb[:, 4:6])
            ms = misc.tile([128, 2], F32, name=f"ms_{b}", tag="ms")
            nc.vector.tensor_mul(ms, mean, s2)
            t2 = misc.tile([128, 2], F32, name=f"t2_{b}", tag="t2")
            nc.vector.tensor_sub(t2, bp_sb[:, 6:8], ms)

            for k in range(2):
                for c in range(2):
                    sl = slice(c * 2048, (c + 1) * 2048)
                    ob = outp.tile([128, 2048], BF16, name=f"ob_{b}_{k}_{c}",
                                   tag="ob")
                    nc.vector.tensor_scalar(
                        ob, xqb[b][:, k, sl],
                        s2[:, k:k + 1], t2[:, k:k + 1], OP.mult, OP.add)
                    nc.sync.dma_start(out=out_d[b][:, k, sl], in_=ob)

        compute(0)
        compute(1)
        finish(0)
        finish(1)

    nc.finalize()
    return nc


def _get_nc():
    if "nc" not in _CACHE:
        _CACHE["nc"] = _build()
    return _CACHE["nc"]


def _pack_host(inputs):
    import ml_dtypes
    bf16 = ml_dtypes.bfloat16
    f8 = ml_dtypes.float8_e4m3
    f32 = np.float32

    def blkfold(a):  # [256, cols] -> [128, 2, cols]
        return np.ascontiguousarray(
            a.reshape(2, 128, -1).transpose(1, 0, 2))

    wqs = []
    wkvs = []
    wps = []
    for b in ("1", "2"):
        wqs.append(blkfold(np.asarray(inputs[f"w_q{b}"], f32).T * WS))
        wkvs.append(blkfold(np.concatenate(
            [np.asarray(inputs[f"w_k{b}"], f32).T,
             np.asarray(inputs[f"w_v{b}"], f32).T], axis=1) * WS))
        wpT = np.ascontiguousarray(np.asarray(inputs[f"w_proj{b}"], f32).T)
        wps.append(wpT[0:128, :])
        wps.append(wpT[128:256, :])
    wq = np.ascontiguousarray(np.concatenate(wqs, axis=2)).astype(f8)
    wkv = np.ascontiguousarray(np.concatenate(wkvs, axis=2)).astype(f8)
    wp = np.ascontiguousarray(np.concatenate(wps, axis=1)).astype(bf16)

    bv1 = np.asarray(inputs["b_v1"], f32) * VS
    bv2 = np.asarray(inputs["b_v2"], f32) * VS
    g = np.asarray(inputs["gamma"], f32)
    be = np.asarray(inputs["beta"], f32)
    bp = np.stack([bv1[:128], bv1[128:], bv2[:128], bv2[128:],
                   g[:128], g[128:], be[:128], be[128:]], axis=1)
    return wq, wkv, wp, np.ascontiguousarray(bp)


def kernel(rgb_low, rgb_high, dsm_low, dsm_high,
           w_q1, b_q1, w_k1, b_k1, w_v1, b_v1,
           w_q2, b_q2, w_k2, b_k2, w_v2, b_v2,
           w_proj1, b_proj1, w_proj2, b_proj2, gamma, beta,
           _trace=False):
    import ml_dtypes
    from concourse.bass_utils import run_bass_kernel_spmd
    bf16 = ml_dtypes.bfloat16
    f8 = ml_dtypes.float8_e4m3
    f32 = np.float32

    inputs = dict(w_q1=w_q1, w_k1=w_k1, w_v1=w_v1, w_proj1=w_proj1,
                  w_q2=w_q2, w_k2=w_k2, w_v2=w_v2, w_proj2=w_proj2,
                  b_v1=b_v1, b_v2=b_v2, gamma=gamma, beta=beta)
    wq, wkv, wp, bp = _pack_host(inputs)
    nc = _get_nc()

    def blkfold(a):  # [256, N] -> [128, 2, N]
        return np.ascontiguousarray(a.reshape(2, 128, N).transpose(1, 0, 2))

    srcs = [(np.asarray(rgb_low, f32), np.asarray(dsm_high, f32)),
            (np.asarray(rgb_high, f32), np.asarray(dsm_low, f32))]
    B = srcs[0][0].shape[0]
    assert B == NCORES, f"expected batch {NCORES}, got {B}"

    in_maps = []
    for i in range(NCORES):
        m = {"wq": wq, "wkv": wkv, "wp": wp, "bp": bp}
        for b in range(2):
            xq = blkfold(srcs[b][0][i].reshape(C, N))
            xkv = blkfold(srcs[b][1][i].reshape(C, N))
            m[f"xq8_{b}"] = xq.astype(f8)
            m[f"xqb_{b}"] = xq.astype(bf16)
            m[f"xkv8_{b}"] = xkv.astype(f8)
        in_maps.append(m)

    res = run_bass_kernel_spmd(nc, in_maps, core_ids=list(range(NCORES)),
                               trace=_trace)

    outs = []
    for b in range(2):
        o = np.stack([
            np.asarray(res.results[i][f"out{b}"], f32)
            .transpose(1, 0, 2).reshape(C, 64, 64)
            for i in range(NCORES)])
        outs.append(o)
    if _trace:
        _CACHE["last_results"] = res
    return (outs[0], outs[1], np.asarray(dsm_low), np.asarray(dsm_high))


# revision 21
# speedup vs baseline: 1.4671x; 1.1064x over previous
# Trainium2 Bass kernel for nn_CrossFrequencyInteraction.
#
# Reference computation (per batch item, two symmetric branches):
#   q = Wq @ x_q;  k = Wk @ x_kv;  v = Wv @ x_kv          (1x1 convs, C=256)
#   out = softmax_n(q) used against ctx = softmax_n(k) @ v^T   (linear attention)
#   inter = Wp @ out;  x_q += inter
#   then training-mode BatchNorm over (B,H,W) on both updated rgb tensors.
#
# Sharding: data-parallel over batch (B=8 -> 1 item per core, 8 cores).
# BN statistics (per-channel sum/sumsq) are AllReduced across cores (2KB).
#
# Numerics: all conv-type matmuls run in fp8e4m3 with DoubleRow perf mode
# (256-deep contraction in a single PE pass at 2x rate). Weights are scaled
# x512 on the host to sit in fp8's normal range; the descale is folded into
# the exp() activation scale (exact, power of two). v is rescaled x32 into
# fp8 for the ctx matmul; the 1/32 is folded into the softmax normalizer.
# The residual x rides in bf16; outputs are stored bf16 and upcast on the
# host. Measured end-to-end absmax-rel ~6e-3 (tolerance 2e-2).
#
# Same algebraic restructurings as the bf16 version (all exact): b_q/b_k
# cancel in softmax, b_proj absorbed by BN, softmax normalizers folded into
# the tiny M = Wp . blockdiag(ctx^T) matrix so attention-out + proj become
# one fp8 matmul per branch.

import os
import numpy as np

C = 256
N = 4096
NCORES = 8
BHW = 8 * 64 * 64
EPS = 1e-5
WS = 512.0     # fp8 weight scale
VS = 32.0      # extra fp8 scale on v going into ctx
MS = 256.0     # fp8 scale on mt (the folded Wp.ctx^T matrix)

_CACHE = {}


def _build():
    import concourse.bass as bass
    import concourse.bacc as bacc
    import concourse.tile as tile
    from concourse import mybir
    from contextlib import ExitStack

    F32 = mybir.dt.float32
    BF16 = mybir.dt.bfloat16
    F8 = mybir.dt.float8e4
    OP = mybir.AluOpType
    AF = mybir.ActivationFunctionType
    AX = mybir.AxisListType
    DR = mybir.MatmulPerfMode.DoubleRow

    nc = bacc.Bacc("TRN2", num_devices=NCORES)

    # inputs, packed on host: [c_lo 128, c_blk 2, n 4096]
    xq8_d = [nc.dram_tensor(f"xq8_{b}", [128, 2, N], F8, kind="ExternalInput")
             for b in range(2)]
    xqb_d = [nc.dram_tensor(f"xqb_{b}", [128, 2, N], BF16, kind="ExternalInput")
             for b in range(2)]
    xkv8_d = [nc.dram_tensor(f"xkv8_{b}", [128, 2, N], F8, kind="ExternalInput")
              for b in range(2)]
    # weights: wq [128,2, b*256+co], wkv [128,2, b*512 + (k 0:256 | v 256:512)]
    wq_d = nc.dram_tensor("wq", [128, 2, 512], F8, kind="ExternalInput")
    wkv_d = nc.dram_tensor("wkv", [128, 2, 1024], F8, kind="ExternalInput")
    # wp: [128, (b*2+p)*256 + co] = WpT_b rows p*128:(p+1)*128
    wp_d = nc.dram_tensor("wp", [128, 1024], BF16, kind="ExternalInput")
    # bp: cols 0:2 bv1*32 (blk), 2:4 bv2*32, 4:6 gamma, 6:8 beta
    bp_d = nc.dram_tensor("bp", [128, 8], F32, kind="ExternalInput")
    out_d = [nc.dram_tensor(f"out{b}", [128, 2, N], BF16, kind="ExternalOutput")
             for b in range(2)]

    with ExitStack() as ctx:
        tc = ctx.enter_context(tile.TileContext(nc))
        const = ctx.enter_context(tc.tile_pool(name="const", bufs=1))
        xp = ctx.enter_context(tc.tile_pool(name="xp", bufs=1))
        ekp = ctx.enter_context(tc.tile_pool(name="ekp", bufs=4))
        eqp = ctx.enter_context(tc.tile_pool(name="eqp", bufs=1))
        misc = ctx.enter_context(tc.tile_pool(name="misc", bufs=1))
        scrp = ctx.enter_context(tc.tile_pool(name="scrp", bufs=2))
        outp = ctx.enter_context(tc.tile_pool(name="outp", bufs=4))
        pkvp = ctx.enter_context(tc.tile_pool(name="pkvp", bufs=3, space="PSUM"))
        qip = ctx.enter_context(tc.tile_pool(name="qip", bufs=2, space="PSUM"))
        ctxp = ctx.enter_context(tc.tile_pool(name="ctxp", bufs=1, space="PSUM"))
        pmtp = ctx.enter_context(tc.tile_pool(name="pmtp", bufs=1, space="PSUM"))
        dramp = ctx.enter_context(tc.tile_pool(name="dramp", bufs=1, space="DRAM"))

        rg = [list(range(NCORES))]

        # ---- constants (sync queue, ahead of inputs) ----
        wkv_sb = const.tile([128, 2, 1024], F8, name="wkv", tag="wkv")
        nc.sync.dma_start(out=wkv_sb, in_=wkv_d[:, :, :])
        wq_sb = const.tile([128, 2, 512], F8, name="wq", tag="wq")
        nc.sync.dma_start(out=wq_sb, in_=wq_d[:, :, :])
        # ---- inputs, issued upfront in consumption order ----
        xkv8 = []
        xq8 = []
        xqb = []
        for b in range(2):
            xkv8.append(xp.tile([128, 2, N], F8, name=f"xkv8_{b}", tag=f"xkv8_{b}"))
            xq8.append(xp.tile([128, 2, N], F8, name=f"xq8_{b}", tag=f"xq8_{b}"))
            xqb.append(xp.tile([128, 2, N], BF16, name=f"xqb_{b}", tag=f"xqb_{b}"))
        for h in range(2):
            sl = slice(h * 2048, (h + 1) * 2048)
            nc.sync.dma_start(out=xkv8[0][:, :, sl], in_=xkv8_d[0][:, :, sl])
        for h in range(2):
            sl = slice(h * 2048, (h + 1) * 2048)
            nc.sync.dma_start(out=xq8[0][:, :, sl], in_=xq8_d[0][:, :, sl])
        wp_sb = const.tile([128, 1024], BF16, name="wp", tag="wp")
        nc.sync.dma_start(out=wp_sb, in_=wp_d[:, :])
        bp_sb = const.tile([128, 8], F32, name="bp", tag="bp")
        nc.sync.dma_start(out=bp_sb, in_=bp_d[:, :])
        for h in range(2):
            sl = slice(h * 2048, (h + 1) * 2048)
            nc.sync.dma_start(out=xqb[0][:, :, sl], in_=xqb_d[0][:, :, sl])
        for b in (1,):
            for h in range(2):
                sl = slice(h * 2048, (h + 1) * 2048)
                nc.sync.dma_start(out=xkv8[b][:, :, sl], in_=xkv8_d[b][:, :, sl])
            for h in range(2):
                sl = slice(h * 2048, (h + 1) * 2048)
                nc.sync.dma_start(out=xq8[b][:, :, sl], in_=xq8_d[b][:, :, sl])
            for h in range(2):
                sl = slice(h * 2048, (h + 1) * 2048)
                nc.sync.dma_start(out=xqb[b][:, :, sl], in_=xqb_d[b][:, :, sl])

        ones8 = const.tile([128, 2, 128], F8, name="ones8", tag="ones8")
        nc.vector.memset(ones8, 1.0)
        epst = const.tile([128, 1], F32, name="epst", tag="epst")
        nc.vector.memset(epst, EPS)

        # per-branch state carried from compute to finish
        st = [{}, {}]

        def compute(b):
            wofs = b * 512
            # ---- KV phase: one DR matmul pass per 128-n tile ----
            pctxf = ctxp.tile([128, 512], F32, name=f"pctx_{b}", tag="pctx")
            pctx = pctxf[:, 0:256]
            pden = pctxf[:, 256:512]
            prev = None
            for p in range(16):
                ek2 = ekp.tile([128, 2, 256], F8, name=f"ek_{b}_{p}", tag="ek")
                vt2 = ekp.tile([128, 2, 256], F8, name=f"vt_{b}_{p}", tag="vt")
                for tt in range(2):
                    t = 2 * p + tt
                    pkv = pkvp.tile([128, 512], F32, name=f"pkv_{b}_{t}", tag="pkv")
                    nc.tensor.matmul(
                        pkv[:, 0:256],
                        lhsT=xkv8[b][:, :, t * 128:(t + 1) * 128],
                        rhs=wkv_sb[:, :, wofs:wofs + 256],
                        start=True, stop=True, perf_mode=DR)
                    nc.tensor.matmul(
                        pkv[:, 256:512],
                        lhsT=xkv8[b][:, :, t * 128:(t + 1) * 128],
                        rhs=wkv_sb[:, :, wofs + 256:wofs + 512],
                        start=True, stop=True, perf_mode=DR)
                    nc.scalar.activation(ek2[:, tt, :], pkv[:, 0:256], AF.Exp,
                                         scale=1.0 / WS)
                    if b == 0:
                        nc.vector.tensor_scalar(vt2[:, tt, :], pkv[:, 256:512],
                                                VS / WS, None, OP.mult)
                    else:
                        nc.scalar.activation(vt2[:, tt, :], pkv[:, 256:512],
                                             AF.Copy, scale=VS / WS)
                if prev is not None:
                    pe, pv, pp = prev
                    for blk in range(2):
                        nc.tensor.matmul(
                            pctx[:, blk * 128:(blk + 1) * 128],
                            lhsT=pv[:, :, blk * 128:(blk + 1) * 128],
                            rhs=pe[:, :, blk * 128:(blk + 1) * 128],
                            start=(pp == 0), stop=False, perf_mode=DR,
                            skip_group_check=True)
                    nc.tensor.matmul(
                        pden, lhsT=ones8, rhs=pe[:, :, :],
                        start=(pp == 0), stop=False, perf_mode=DR,
                        skip_group_check=True)
                prev = (ek2, vt2, p)
            pe, pv, pp = prev
            for blk in range(2):
                nc.tensor.matmul(
                    pctx[:, blk * 128:(blk + 1) * 128],
                    lhsT=pv[:, :, blk * 128:(blk + 1) * 128],
                    rhs=pe[:, :, blk * 128:(blk + 1) * 128],
                    start=False, stop=True, perf_mode=DR,
                    skip_group_check=True)
            nc.tensor.matmul(
                pden, lhsT=ones8, rhs=pe[:, :, :],
                start=False, stop=True, perf_mode=DR,
                skip_group_check=True)

            # ---- ctx evict (+ scaled b_v fold) and k-softmax denominators ----
            ctx2 = misc.tile([128, 256], BF16, name=f"ctx2_{b}", tag=f"ctx2_{b}")
            for blk in range(2):
                nc.vector.tensor_scalar(
                    ctx2[:, blk * 128:(blk + 1) * 128],
                    pctx[:, blk * 128:(blk + 1) * 128],
                    bp_sb[:, 2 * b + blk:2 * b + blk + 1], None, OP.add)
            drow = misc.tile([1, 256], F32, name=f"drow_{b}", tag="drow")
            nc.vector.tensor_copy(drow, pden[0:1, :])
            denT = misc.tile([128, 2], F32, name=f"denT_{b}", tag="denT")
            for blk in range(2):
                nc.sync.dma_start(out=denT[:, blk:blk + 1],
                                  in_=drow[0:1, blk * 128:(blk + 1) * 128])

            # ---- Q phase: DR conv + exp with accumulated row sums ----
            expq8 = eqp.tile([128, 2, N], F8, name=f"expq_{b}", tag=f"expq_{b}")
            sqp = misc.tile([128, 2, 8], F32, name=f"sqp_{b}", tag="sqp")
            for k in range(2):
                for jp in range(8):
                    pq2 = qip.tile([128, 512], F32, name=f"pq_{b}_{k}_{jp}",
                                   tag="mm256")
                    for tj in range(2):
                        j = 2 * jp + tj
                        nc.tensor.matmul(
                            pq2[:, tj * 256:(tj + 1) * 256],
                            lhsT=wq_sb[:, :,
                                       b * 256 + k * 128:b * 256 + (k + 1) * 128],
                            rhs=xq8[b][:, :, j * 256:(j + 1) * 256],
                            start=True, stop=True, perf_mode=DR)
                    nc.scalar.activation(
                        expq8[:, k, jp * 512:(jp + 1) * 512],
                        pq2, AF.Exp,
                        scale=1.0 / WS, accum_out=sqp[:, k, jp:jp + 1])

            # ---- normalization: fac = 8 / (den_k * sum_q)  (8 = 256/32) ----
            sq2 = misc.tile([128, 2], F32, name=f"sq2_{b}", tag="sq2")
            for k in range(2):
                nc.vector.reduce_sum(sq2[:, k:k + 1], sqp[:, k, :], axis=AX.X)
            fde = misc.tile([128, 2], F32, name=f"fde_{b}", tag="fde")
            nc.vector.tensor_scalar(fde, sq2, MS / VS, None, OP.mult)
            nc.vector.tensor_mul(fde, fde, denT)
            fac = misc.tile([128, 2], F32, name=f"fac_{b}", tag="fac")
            nc.vector.reciprocal(fac, fde)

            # ---- M^T: per-head Wp . ctx^T with folded normalization ----
            mt8 = misc.tile([128, 2, 256], F8, name=f"mt_{b}", tag=f"mt_{b}")
            pmtf = pmtp.tile([128, 512], F32, name=f"pmt_{b}", tag="pmt")
            for p in range(2):
                pmt = pmtf[:, p * 256:(p + 1) * 256]
                for hh in range(2):
                    h = 2 * p + hh
                    nc.tensor.matmul(
                        pmt[hh * 64:(hh + 1) * 64, :],
                        lhsT=ctx2[hh * 64:(hh + 1) * 64, h * 64:(h + 1) * 64],
                        rhs=wp_sb[hh * 64:(hh + 1) * 64,
                                  (b * 2 + p) * 256:(b * 2 + p + 1) * 256],
                        start=True, stop=True)
                nc.vector.tensor_scalar(mt8[:, p, :], pmt, fac[:, p:p + 1],
                                        None, OP.mult)

            # ---- inter = M^T.T @ expq (fp8 DR), residual + stats on gpsimd ----
            sumr = misc.tile([128, 2, 8], F32, name=f"sumr_{b}", tag="sumr")
            ssq = misc.tile([128, 2, 4], F32, name=f"ssq_{b}", tag="ssq")
            for k in range(2):
                for jp in range(8):
                    pi2 = qip.tile([128, 512], F32, name=f"pi_{b}_{k}_{jp}",
                                   tag="mm256")
                    for tj in range(2):
                        j = 2 * jp + tj
                        nc.tensor.matmul(
                            pi2[:, tj * 256:(tj + 1) * 256],
                            lhsT=mt8[:, :, k * 128:(k + 1) * 128],
                            rhs=expq8[:, :, j * 256:(j + 1) * 256],
                            start=True, stop=True, perf_mode=DR)
                    xs2 = xqb[b][:, k, jp * 512:(jp + 1) * 512]
                    nc.vector.scalar_tensor_tensor(
                        xs2, pi2, 1.0 / MS, xs2,
                        OP.mult, OP.add, accum_out=sumr[:, k, jp:jp + 1])
                    if jp % 2 == 1:
                        c = jp // 2
                        xc = xqb[b][:, k, c * 1024:(c + 1) * 1024]
                        scr = scrp.tile([128, 1024], F32,
                                        name=f"scr_{b}_{k}_{c}", tag="scr")
                        nc.gpsimd.tensor_tensor(scr, xc, xc, OP.mult)
                        nc.vector.reduce_sum(ssq[:, k, c:c + 1], scr, axis=AX.X)

            # ---- pack per-core sums and AllReduce ----
            cc_sb = misc.tile([128, 4], F32, name=f"cc_{b}", tag=f"cc{b}")
            for k in range(2):
                nc.vector.reduce_sum(cc_sb[:, 2 * k:2 * k + 1],
                                     sumr[:, k, :], axis=AX.X)
                nc.vector.reduce_sum(cc_sb[:, 2 * k + 1:2 * k + 2],
                                     ssq[:, k, :], axis=AX.X)
            cc_in = dramp.tile([128, 4], F32, name=f"ccin_{b}", tag=f"ccin{b}")
            cc_out = dramp.tile([128, 4], F32, name=f"ccout_{b}", tag=f"ccout{b}",
                                addr_space="Shared")
            nc.sync.dma_start(out=cc_in, in_=cc_sb)
            if os.environ.get("KERNEL_ABLATE") == "nocc":
                ccr = misc.tile([128, 4], F32, name=f"ccr_{b}", tag=f"ccr{b}")
                nc.vector.tensor_scalar(ccr, cc_sb, float(NCORES), None, OP.mult)
                st[b]["ccr"] = ccr
            else:
                nc.gpsimd.collective_compute(
                    "AllReduce", OP.add, replica_groups=rg,
                    ins=[cc_in[:, :]], outs=[cc_out[:, :]])
                st[b]["cc_out"] = cc_out

        def finish(b):
            if "cc_out" in st[b]:
                ccr = misc.tile([128, 4], F32, name=f"ccr_{b}", tag=f"ccr{b}")
                nc.gpsimd.dma_start(out=ccr, in_=st[b]["cc_out"])
            else:
                ccr = st[b]["ccr"]
            ccr3 = ccr.rearrange("p (k s) -> p k s", k=2)
            mean = misc.tile([128, 2], F32, name=f"mean_{b}", tag="mean")
            nc.vector.tensor_scalar(mean, ccr3[:, :, 0], 1.0 / BHW, None, OP.mult)
            m2 = misc.tile([128, 2], F32, name=f"m2_{b}", tag="m2")
            nc.vector.tensor_mul(m2, mean, mean)
            var = misc.tile([128, 2], F32, name=f"var_{b}", tag="var")
            nc.vector.scalar_tensor_tensor(
                var, ccr3[:, :, 1], 1.0 / BHW, m2, OP.mult, OP.subtract)
            sd = misc.tile([128, 2], F32, name=f"sd_{b}", tag="sd")
            nc.scalar.activation(sd, var, AF.Sqrt, bias=epst)
            rs = misc.tile([128, 2], F32, name=f"rs_{b}", tag="rs")
            nc.vector.reciprocal(rs, sd)
            s2 = misc.tile([128, 2], F32, name=f"s2_{b}", tag="s2")
            nc.vector.tensor_mul(s2, rs, bp_sb[:, 4:6])
            ms = misc.tile([128, 2], F32, name=f"ms_{b}", tag="ms")
            nc.vector.tensor_mul(ms, mean, s2)
            t2 = misc.tile([128, 2], F32, name=f"t2_{b}", tag="t2")
            nc.vector.tensor_sub(t2, bp_sb[:, 6:8], ms)

            for k in range(2):
                for c in range(2):
                    sl = slice(c * 2048, (c + 1) * 2048)
                    ob = outp.tile([128, 2048], BF16, name=f"ob_{b}_{k}_{c}",
                                   tag="ob")
                    nc.vector.tensor_scalar(
                        ob, xqb[b][:, k, sl],
                        s2[:, k:k + 1], t2[:, k:k + 1], OP.mult, OP.add)
                    nc.sync.dma_start(out=out_d[b][:, k, sl], in_=ob)

        compute(0)
        compute(1)
        finish(0)
        finish(1)

    nc.finalize()
    return nc


def _get_nc():
    if "nc" not in _CACHE:
        _CACHE["nc"] = _build()
    return _CACHE["nc"]


def _pack_host(inputs):
    import ml_dtypes
    bf16 = ml_dtypes.bfloat16
    f8 = ml_dtypes.float8_e4m3
    f32 = np.float32

    def blkfold(a):  # [256, cols] -> [128, 2, cols]
        return np.ascontiguousarray(
            a.reshape(2, 128, -1).transpose(1, 0, 2))

    wqs = []
    wkvs = []
    wps = []
    for b in ("1", "2"):
        wqs.append(blkfold(np.asarray(inputs[f"w_q{b}"], f32).T * WS))
        wkvs.append(blkfold(np.concatenate(
            [np.asarray(inputs[f"w_k{b}"], f32).T,
             np.asarray(inputs[f"w_v{b}"], f32).T], axis=1) * WS))
        wpT = np.ascontiguousarray(np.asarray(inputs[f"w_proj{b}"], f32).T)
        wps.append(wpT[0:128, :])
        wps.append(wpT[128:256, :])
    wq = np.ascontiguousarray(np.concatenate(wqs, axis=2)).astype(f8)
    wkv = np.ascontiguousarray(np.concatenate(wkvs, axis=2)).astype(f8)
    wp = np.ascontiguousarray(np.concatenate(wps, axis=1)).astype(bf16)

    bv1 = np.asarray(inputs["b_v1"], f32) * VS
    bv2 = np.asarray(inputs["b_v2"], f32) * VS
    g = np.asarray(inputs["gamma"], f32)
    be = np.asarray(inputs["beta"], f32)
    bp = np.stack([bv1[:128], bv1[128:], bv2[:128], bv2[128:],
                   g[:128], g[128:], be[:128], be[128:]], axis=1)
    return wq, wkv, wp, np.ascontiguousarray(bp)


def kernel(rgb_low, rgb_high, dsm_low, dsm_high,
           w_q1, b_q1, w_k1, b_k1, w_v1, b_v1,
           w_q2, b_q2, w_k2, b_k2, w_v2, b_v2,
           w_proj1, b_proj1, w_proj2, b_proj2, gamma, beta,
           _trace=False):
    import ml_dtypes
    from concourse.bass_utils import run_bass_kernel_spmd
    bf16 = ml_dtypes.bfloat16
    f8 = ml_dtypes.float8_e4m3
    f32 = np.float32

    inputs = dict(w_q1=w_q1, w_k1=w_k1, w_v1=w_v1, w_proj1=w_proj1,
                  w_q2=w_q2, w_k2=w_k2, w_v2=w_v2, w_proj2=w_proj2,
                  b_v1=b_v1, b_v2=b_v2, gamma=gamma, beta=beta)
    wq, wkv, wp, bp = _pack_host(inputs)
    nc = _get_nc()

    def blkfold(a):  # [256, N] -> [128, 2, N]
        return np.ascontiguousarray(a.reshape(2, 128, N).transpose(1, 0, 2))

    srcs = [(np.asarray(rgb_low, f32), np.asarray(dsm_high, f32)),
            (np.asarray(rgb_high, f32), np.asarray(dsm_low, f32))]
    B = srcs[0][0].shape[0]
    assert B == NCORES, f"expected batch {NCORES}, got {B}"

    in_maps = []
    for i in range(NCORES):
        m = {"wq": wq, "wkv": wkv, "wp": wp, "bp": bp}
        for b in range(2):
            xq = blkfold(srcs[b][0][i].reshape(C, N))
            xkv = blkfold(srcs[b][1][i].reshape(C, N))
            m[f"xq8_{b}"] = xq.astype(f8)
            m[f"xqb_{b}"] = xq.astype(bf16)
            m[f"xkv8_{b}"] = xkv.astype(f8)
        in_maps.append(m)

    res = run_bass_kernel_spmd(nc, in_maps, core_ids=list(range(NCORES)),
                               trace=_trace)

    outs = []
    for b in range(2):
        o = np.stack([
            np.asarray(res.results[i][f"out{b}"], f32)
            .transpose(1, 0, 2).reshape(C, 64, 64)
            for i in range(NCORES)])
        outs.append(o)
    if _trace:
        _CACHE["last_results"] = res
    return (outs[0], outs[1], np.asarray(dsm_low), np.asarray(dsm_high))
